# revision 9
# baseline (speedup 1.0000x reference)
"""MoE (DeepSeek-style naive top-k routing + per-expert SwiGLU) on 8 Trainium2 cores.

Strategy: expert parallelism with host-side token dispatch/combine.
  - Host computes the routing (top_k_index/top_k_weights -> per-expert token
    lists + combine gates), gathers each expert's tokens into a padded
    capacity-C buffer, and hands core e exactly expert e's weights + tokens.
  - Each core runs dense SwiGLU over its C tokens in bf16 (1 col/cycle on the
    PE, same rate as f32r, but half the HBM traffic):
        Y^T = W12^T @ X^T           (GEMM1, contraction over DIM=1024,
                                     11 unpadded 128-row output chunks)
        hidden = silu(x1) * x2      (partition-shifted 64-wide muls: x2
                                     chunks are offset by 64 partitions
                                     from x1 chunks since H=704=5.5*128)
        out = hidden^T' @ W3        (GEMM2, contraction over H in 6 chunks,
                                     last chunk 64 rows)
    with the per-token combine gate folded into the PSUM->SBUF copy of the
    GEMM2 result.
  - Host scatter-adds the 8 per-expert partial outputs into the [T, DIM] out.

Ramp optimizations: DMA triggers alternate between the two HW-DGE rings
(Sync + Activation engines) so descriptor generation is not serialized; the
first w12/xT chunks are split small so the PE starts ~6us earlier; a short
burst of warm-up matmuls on zeroed SBUF ramps the PE clock (DVFS) while the
first DMAs are still in flight.
"""

import os
import sys

for _p in ("/opt/trn_rl_repo",):
    if _p not in sys.path:
        sys.path.insert(0, _p)

import numpy as np

E = 8
DIM = 1024
H = 704
TOPK = 2
KD = DIM // 128      # contraction tiles for GEMM1
NCH = (2 * H) // 128  # GEMM1 output chunks (11, unpadded)
KH = (H + 127) // 128  # GEMM2 contraction chunks (6, last is 64 rows)
NP = H // 128         # full swiglu pairs (5); pair 5 is the 64-wide tail
DSLICE = 512          # DIM slice width for GEMM2
N_CORES = 8
WARM_N = 8            # PE warm-up matmuls (DVFS ramp) before data lands


def _token_slices(C):
    """Split C (mult of 128) into GEMM1 slice widths <=512, each >=256
    where possible (f32r runs 1 cyc/row only at N>=256)."""
    out = []
    rem = C
    while rem > 640:
        out.append(512)
        rem -= 512
    if rem > 512:
        a = (rem // 2 + 127) // 128 * 128
        out += [a, rem - a]
    elif rem:
        out.append(rem)
    return out


MM_DT_NAME = os.environ.get("KERNEL_MM_DT", "bf16")  # f32 | f32r | bf16

_BUILD_CACHE = {}
LAST_RESULTS = None  # test harness reads exec_time_ns etc. from here


def _ensure_ntff_hook():
    """Profiling-only: register the ctypes NTFF hook (antenv.axon_hooks is
    not shipped in this container) and keep profile post-processing local."""
    import types

    import concourse.bass_utils as bu

    try:
        from antenv.axon_hooks import get_axon_ntff_profile_hook  # noqa: F401
    except ImportError:
        try:
            from trn_agent_boot.trn_boot import _ntff_profile_via_ctypes

            hook = _ntff_profile_via_ctypes("/opt/axon/libaxon_pjrt.so")
        except Exception:
            hook = None
        mod = types.ModuleType("antenv.axon_hooks")
        mod.get_axon_ntff_profile_hook = lambda: hook
        mod.set_axon_ntff_profile_hook = lambda h: None
        sys.modules["antenv.axon_hooks"] = mod
        import antenv

        antenv.axon_hooks = mod
    # keep artifacts local — no bucket in this container
    bu.upload_artifacts = lambda tmpdir: f"local://{tmpdir}"


def _install_drain_patch():
    """walrus 2026-05 rejects >1 sem wait on CTRL-class (Drain/NoOp) SP
    instructions; respell Tile's tail drain as a chain of 1-wait NOPs."""
    import concourse.mybir as mybir
    import concourse.tile as tile
    from concourse.tile import ScopedClock

    if getattr(tile.TileContext, "_drain_patch_installed", False):
        return

    def _patched(self, tick_clock, wait_clock):
        nc = self.nc
        nop_inst = nc.sync.nop(nofuse=True, hint="drain_waits")
        wait_clock.add_sem_waits(
            nop_inst.ins, ScopedClock({None: tick_clock.global_clock})
        )
        waits = list(nop_inst.ins.sync_info.on_wait or [])
        if len(waits) > 1:
            nop_inst.ins.sync_info.on_wait = waits[:1]
            for w in waits[1:]:
                extra = nc.sync.nop(nofuse=True, hint="drain_waits")
                extra.ins.sync_info = mybir.SyncInfo(on_wait=[w], on_update=[])
        nc.sync.drain()
        nc.all_engine_barrier()
        assert self.sems is not None
        popped = nc._tile_sem_poison_stack.pop()
        assert popped is self._sem_poison
        nc.clear_and_free_semaphores(list(self.sems.allocated().values()))
        nc.all_engine_barrier()

    tile.TileContext._drain_and_barrier = _patched
    tile.TileContext._drain_patch_installed = True


def _build_program(C, mm_dt, with_b12):
    """Build the single-core Bass program (SPMD: same program, per-core data)."""
    import concourse.bacc as bacc
    import concourse.bass as bass  # noqa: F401
    import concourse.mybir as mybir
    import concourse.tile as tile

    f32 = mybir.dt.float32
    if mm_dt == "bf16":
        io_dt = mybir.dt.bfloat16
        out_dt = mybir.dt.bfloat16
    elif mm_dt == "f32r":
        io_dt = mybir.dt.float32r
        out_dt = f32
    else:
        io_dt = f32
        out_dt = f32

    SL = _token_slices(C)
    SOFF = [0]
    for w in SL:
        SOFF.append(SOFF[-1] + w)
    TN = len(SL)
    NT = C // 128        # token tiles for GEMM2

    nc = bacc.Bacc("TRN2", target_bir_lowering=False, debug=False,
                   enable_asserts=False, num_devices=N_CORES)

    # Host-packed partition-major layouts: every DMA below moves full
    # contiguous per-partition rows.
    xT = nc.dram_tensor("xT", [128, KD * C], io_dt, kind="ExternalInput")
    w12 = nc.dram_tensor("w12", [128, NCH * KD * 128], io_dt,
                         kind="ExternalInput")
    w3 = nc.dram_tensor("w3", [128, KH * DIM], io_dt, kind="ExternalInput")
    gt = nc.dram_tensor("gt", [128, NT], f32, kind="ExternalInput")
    if with_b12:
        b1 = nc.dram_tensor("b1", [128, KH], f32, kind="ExternalInput")
        b2 = nc.dram_tensor("b2", [128, KH], f32, kind="ExternalInput")
    out = nc.dram_tensor("out", [C, DIM], out_dt, kind="ExternalOutput")

    silu = mybir.ActivationFunctionType.Silu

    with tile.TileContext(nc) as tc:
        with (
            tc.tile_pool(name="weights", bufs=1) as wpool,
            tc.tile_pool(name="tmp", bufs=4) as tpool,
            tc.tile_pool(name="ps_g1", bufs=3, space="PSUM") as pspool1,
            tc.tile_pool(name="ps_g2", bufs=4, space="PSUM") as pspool2,
        ):
            w12sb = wpool.tile([128, NCH, KD, 128], io_dt, tag="w12sb")
            xTsb = wpool.tile([128, KD * C], io_dt, tag="xTsb")
            w3sb = wpool.tile([128, KH, DIM], io_dt, tag="w3sb")
            gsb = wpool.tile([128, NT], f32, tag="gsb")
            hid = wpool.tile([128, KH, C], io_dt, tag="hid")
            ssb = wpool.tile([128, KH, 512], f32, tag="ssb")
            warm = wpool.tile([128, 384], io_dt, tag="warm")
            if with_b12:
                b1sb = wpool.tile([128, KH], f32, tag="b1sb")
                b2sb = wpool.tile([128, KH], f32, tag="b2sb")

            # ---- PE warm-up: ramp DVFS while the first DMAs are in flight
            nc.vector.memset(warm[:], 0)
            wp = pspool2.tile([128, DSLICE], f32, tag="pso",
                              name="wp")[:, :256]
            for i in range(WARM_N):
                nc.tensor.matmul(wp, warm[:, 256:384], warm[:, 0:256],
                                 start=(i == 0), stop=(i == WARM_N - 1))
            # consume so nothing upstream can be considered dead
            wsink = tpool.tile([128, 256], f32, tag="wsink")
            nc.vector.tensor_scalar_mul(wsink, wp, 0.0)

            # ---- input DMAs: two HWDGE rings (sync, scalar) in parallel,
            # few large transfers with fat descriptors (>=2KB/partition-row;
            # small rows are ~80ns/descriptor overhead-bound), ordered by
            # first consumption.
            def w12_dma(eng, c0, c1):
                eng.dma_start(w12sb[:, c0:c1, :, :],
                              w12[:, c0 * KD * 128:c1 * KD * 128])

            def xT_dma(eng, n, k0, k1):
                sw = SL[n]
                base = SOFF[n] * KD
                eng.dma_start(
                    xTsb[:, base + k0 * sw:base + k1 * sw],
                    xT[:, base + k0 * sw:base + k1 * sw])

            sy, sc = nc.sync, nc.scalar
            w12_dma(sy, 0, 1)
            xT_dma(sc, 0, 0, KD // 2)
            w12_dma(sy, 1, 3)
            xT_dma(sc, 0, KD // 2, KD)
            w12_dma(sy, 3, NCH)
            for n in range(1, TN):
                xT_dma(sc, n, 0, KD)
            sc.dma_start(w3sb[:], w3[:])
            sy.dma_start(gsb[:], gt[:])
            if with_b12:
                sc.dma_start(b1sb[:], b1[:])
                sc.dma_start(b2sb[:], b2[:])

            def _gemm2_tile(t):
                tsl = slice(t * 128, (t + 1) * 128)
                for d in range(DIM // DSLICE):
                    pso = pspool2.tile([128, DSLICE], f32, tag="pso")
                    dsl = slice(d * DSLICE, (d + 1) * DSLICE)
                    for k in range(KH):
                        if k == KH - 1 and H % 128:
                            hh = H % 128
                            nc.tensor.matmul(
                                pso, hid[0:hh, k, tsl], w3sb[0:hh, k, dsl],
                                start=(k == 0), stop=True)
                        else:
                            nc.tensor.matmul(
                                pso, hid[:, k, tsl], w3sb[:, k, dsl],
                                start=(k == 0), stop=(k == KH - 1))
                    o = tpool.tile([128, DSLICE], out_dt, tag="o")
                    nc.vector.tensor_scalar_mul(o, pso, gsb[:, t:t + 1])
                    nc.sync.dma_start(out[tsl, dsl], o)

            t_emitted = 0
            for n in range(TN):
                w = SL[n]
                ns = slice(SOFF[n], SOFF[n] + w)
                xbase = SOFF[n] * KD
                # GEMM1: 11 unpadded chunks; chunks 0..5(:64) are x1,
                # chunks 5(64:)..10 are x2, offset by 64 partitions.
                for c in range(NCH):
                    ps = pspool1.tile([128, 512], f32, tag="g1ps",
                                      name="g1ps")[:, :w]
                    for k in range(KD):
                        nc.tensor.matmul(
                            ps, w12sb[:, c, k, :],
                            xTsb[:, xbase + k * w:xbase + (k + 1) * w],
                            start=(k == 0), stop=(k == KD - 1))
                    if c < NP:
                        if with_b12:
                            nc.scalar.activation(ssb[:, c, :w], ps, silu,
                                                 bias=b1sb[:, c:c + 1])
                        else:
                            nc.scalar.activation(ssb[:, c, :w], ps, silu)
                    elif c == NP:
                        # lower 64: x1 tail; upper 64: x2 cols 0..63
                        if with_b12:
                            nc.scalar.activation(ssb[0:64, NP, :w], ps[0:64],
                                                 silu, bias=b1sb[0:64, NP:NP + 1])
                            nc.vector.tensor_scalar_add(
                                ps[64:128], ps[64:128], b2sb[64:128, 0:1])
                        else:
                            nc.scalar.activation(ssb[0:64, NP, :w], ps[0:64],
                                                 silu)
                        nc.vector.tensor_mul(
                            out=hid[0:64, 0, ns], in0=ssb[0:64, 0, :w],
                            in1=ps[64:128])
                    else:
                        p_lo = c - NP - 1   # pair completing its upper half
                        p_hi = c - NP       # pair starting its lower half
                        if with_b12:
                            nc.vector.tensor_scalar_add(
                                ps, ps, b2sb[:, c - NP:c - NP + 1])
                        nc.vector.tensor_mul(
                            out=hid[64:128, p_lo, ns],
                            in0=ssb[64:128, p_lo, :w], in1=ps[0:64])
                        nc.vector.tensor_mul(
                            out=hid[0:64, p_hi, ns],
                            in0=ssb[0:64, p_hi, :w], in1=ps[64:128])

                # GEMM2 lags one slice behind GEMM1 so the PE never waits
                # on the SwiGLU chain at a slice seam.
                for t in range(t_emitted, SOFF[n] // 128):
                    _gemm2_tile(t)
                t_emitted = SOFF[n] // 128
            for t in range(t_emitted, NT):
                _gemm2_tile(t)

    nc.compile()
    return nc


def _np_io_dtype(mm_dt):
    if mm_dt == "bf16":
        import ml_dtypes

        return np.dtype(ml_dtypes.bfloat16)
    return np.dtype(np.float32)


def kernel(hidden_states, top_k_weights, W12, b12, W3, b3, top_k_index):
    global LAST_RESULTS
    from concourse.bass_utils import run_bass_kernel_spmd

    hs = np.asarray(hidden_states, dtype=np.float32)
    wts = np.asarray(top_k_weights, dtype=np.float32)
    idx = np.asarray(top_k_index)
    W12n = np.asarray(W12, dtype=np.float32)
    b12n = np.asarray(b12, dtype=np.float32)
    W3n = np.asarray(W3, dtype=np.float32)
    b3n = np.asarray(b3, dtype=np.float32)

    T = hs.shape[0]
    mm_dt = MM_DT_NAME
    io_np = _np_io_dtype(mm_dt)

    # ---- routing on host ----
    gates = np.zeros((E, T), np.float32)
    for k in range(TOPK):
        np.add.at(gates, (idx[:, k], np.arange(T)), wts[:, k])
    tok = [np.nonzero((idx == e).any(axis=1))[0] for e in range(E)]
    maxlen = max(256, max(len(t) for t in tok))
    C = ((maxlen + 127) // 128) * 128
    NT = C // 128

    with_b12 = bool(np.any(b12n))
    key = (C, mm_dt, with_b12)
    if key not in _BUILD_CACHE:
        _BUILD_CACHE[key] = _build_program(C, mm_dt, with_b12)
    nc = _BUILD_CACHE[key]

    # ---- per-core inputs ----
    in_maps = []
    for e in range(E):
        te = tok[e]
        ne = len(te)
        X = np.zeros((C, DIM), np.float32)
        X[:ne] = hs[te]
        # per-slice [128, KD, w] partition-major packs, concatenated
        blocks = []
        off = 0
        for w in _token_slices(C):
            blk = X[off:off + w].reshape(w, KD, 128).transpose(2, 1, 0)
            blocks.append(np.ascontiguousarray(blk).reshape(128, -1))
            off += w
        xTp = np.concatenate(blocks, axis=1).astype(io_np, copy=False)

        # w12: [DIM, 2H] -> [128, (c, k, m)] with c over 11 output chunks
        w12p = np.ascontiguousarray(
            W12n[e].reshape(KD, 128, NCH, 128).transpose(1, 2, 0, 3)
        ).reshape(128, -1)

        # w3: [H, DIM] zero-padded to 6*128 rows -> [128, (c, d)]
        w3p = np.zeros((KH * 128, DIM), np.float32)
        w3p[:H] = W3n[e]
        w3p = np.ascontiguousarray(
            w3p.reshape(KH, 128, DIM).transpose(1, 0, 2)).reshape(128, -1)

        g = np.zeros((C,), np.float32)
        g[:ne] = gates[e, te]
        gtile = np.ascontiguousarray(g.reshape(NT, 128).T)

        m = {
            "xT": xTp,
            "w12": w12p.astype(io_np, copy=False),
            "w3": w3p.astype(io_np, copy=False),
            "gt": gtile,
        }
        if with_b12:
            b1p = np.zeros((128, KH), np.float32)
            b2p = np.zeros((128, KH), np.float32)
            for c in range(KH):
                n1 = min(128, H - c * 128)
                b1p[:n1, c] = b12n[e][c * 128:c * 128 + n1]
                for p in range(128):
                    j = c * 128 + p - 64
                    if 0 <= j < H:
                        b2p[p, c] = b12n[e][H + j]
            m["b1"] = np.ascontiguousarray(b1p)
            m["b2"] = np.ascontiguousarray(b2p)
        in_maps.append(m)

    trace = bool(os.environ.get("KERNEL_TRACE"))
    kw = {}
    if trace:
        _ensure_ntff_hook()
        kw = {"trace_cores": list(range(N_CORES)), "stitch_traces": False}
    res = run_bass_kernel_spmd(nc, in_maps, list(range(N_CORES)), trace=trace, **kw)
    LAST_RESULTS = res

    # ---- combine on host ----
    out = np.zeros((T, DIM), np.float32)
    for e in range(E):
        te = tok[e]
        out[te] += res.results[e]["out"][:len(te)].astype(np.float32)
    if np.any(b3n):
        out += gates.T @ b3n
    return out


# revision 29
# speedup vs baseline: 1.0309x; 1.0309x over previous
"""MoE (DeepSeek-style naive top-k routing + per-expert SwiGLU) on 8 Trainium2 cores.

Strategy: expert parallelism with host-side token dispatch/combine.
  - Host computes the routing (top_k_index/top_k_weights -> per-expert token
    lists + combine gates), gathers each expert's tokens into a padded
    capacity-C buffer, and hands core e exactly expert e's weights + tokens.
  - Each core runs dense SwiGLU over its C tokens in bf16 (1 col/cycle on the
    PE, same rate as f32r, but half the HBM traffic):
        Y^T = W12^T @ X^T           (GEMM1, contraction over DIM=1024,
                                     11 unpadded 128-row output chunks)
        hidden = silu(x1) * x2      (partition-shifted 64-wide muls: x2
                                     chunks are offset by 64 partitions
                                     from x1 chunks since H=704=5.5*128)
        out = hidden^T' @ W3        (GEMM2, contraction over H in 6 chunks,
                                     last chunk 64 rows)
    with the per-token combine gate folded into the PSUM->SBUF copy of the
    GEMM2 result.
  - Host scatter-adds the 8 per-expert partial outputs into the [T, DIM] out.

Ramp optimizations: DMA triggers alternate between the two HW-DGE rings
(Sync + Activation engines) so descriptor generation is not serialized; the
first w12/xT chunks are split small so the PE starts ~6us earlier; a short
burst of warm-up matmuls on zeroed SBUF ramps the PE clock (DVFS) while the
first DMAs are still in flight.
"""

import os
import sys

for _p in ("/opt/trn_rl_repo",):
    if _p not in sys.path:
        sys.path.insert(0, _p)

import numpy as np

E = 8
DIM = 1024
H = 704
TOPK = 2
KD = DIM // 128      # contraction tiles for GEMM1
NCH = (2 * H) // 128  # GEMM1 output chunks (11, unpadded)
KH = (H + 127) // 128  # GEMM2 contraction chunks (6, last is 64 rows)
NP = H // 128         # full swiglu pairs (5); pair 5 is the 64-wide tail
DSLICE = 512          # DIM slice width for GEMM2
N_CORES = 8
WARM_N = 12           # PE warm-up matmuls (DVFS ramp) before data lands


def _token_slices(C):
    """Split C (mult of 128) into GEMM1 slice widths <=512, each >=256
    where possible (f32r runs 1 cyc/row only at N>=256)."""
    out = []
    rem = C
    while rem > 640:
        out.append(512)
        rem -= 512
    if rem > 512:
        a = (rem // 2 + 127) // 128 * 128
        out += [a, rem - a]
    elif rem:
        out.append(rem)
    return out


MM_DT_NAME = os.environ.get("KERNEL_MM_DT", "bf16")  # f32 | f32r | bf16

_BUILD_CACHE = {}
LAST_RESULTS = None  # test harness reads exec_time_ns etc. from here


def _ensure_ntff_hook():
    """Profiling-only: register the ctypes NTFF hook (antenv.axon_hooks is
    not shipped in this container) and keep profile post-processing local."""
    import types

    import concourse.bass_utils as bu

    try:
        from antenv.axon_hooks import get_axon_ntff_profile_hook  # noqa: F401
    except ImportError:
        try:
            from trn_agent_boot.trn_boot import _ntff_profile_via_ctypes

            hook = _ntff_profile_via_ctypes("/opt/axon/libaxon_pjrt.so")
        except Exception:
            hook = None
        mod = types.ModuleType("antenv.axon_hooks")
        mod.get_axon_ntff_profile_hook = lambda: hook
        mod.set_axon_ntff_profile_hook = lambda h: None
        sys.modules["antenv.axon_hooks"] = mod
        import antenv

        antenv.axon_hooks = mod
    # keep artifacts local — no bucket in this container
    bu.upload_artifacts = lambda tmpdir: f"local://{tmpdir}"


def _install_drain_patch():
    """walrus 2026-05 rejects >1 sem wait on CTRL-class (Drain/NoOp) SP
    instructions; respell Tile's tail drain as a chain of 1-wait NOPs."""
    import concourse.mybir as mybir
    import concourse.tile as tile
    from concourse.tile import ScopedClock

    if getattr(tile.TileContext, "_drain_patch_installed", False):
        return

    def _patched(self, tick_clock, wait_clock):
        nc = self.nc
        nop_inst = nc.sync.nop(nofuse=True, hint="drain_waits")
        wait_clock.add_sem_waits(
            nop_inst.ins, ScopedClock({None: tick_clock.global_clock})
        )
        waits = list(nop_inst.ins.sync_info.on_wait or [])
        if len(waits) > 1:
            nop_inst.ins.sync_info.on_wait = waits[:1]
            for w in waits[1:]:
                extra = nc.sync.nop(nofuse=True, hint="drain_waits")
                extra.ins.sync_info = mybir.SyncInfo(on_wait=[w], on_update=[])
        nc.sync.drain()
        nc.all_engine_barrier()
        assert self.sems is not None
        popped = nc._tile_sem_poison_stack.pop()
        assert popped is self._sem_poison
        nc.clear_and_free_semaphores(list(self.sems.allocated().values()))
        nc.all_engine_barrier()

    tile.TileContext._drain_and_barrier = _patched
    tile.TileContext._drain_patch_installed = True


def _build_program(C, mm_dt, with_b12):
    """Build the single-core Bass program (SPMD: same program, per-core data)."""
    import concourse.bacc as bacc
    import concourse.bass as bass  # noqa: F401
    import concourse.mybir as mybir
    import concourse.tile as tile

    f32 = mybir.dt.float32
    if mm_dt == "bf16":
        io_dt = mybir.dt.bfloat16
        out_dt = mybir.dt.bfloat16
    elif mm_dt == "f32r":
        io_dt = mybir.dt.float32r
        out_dt = f32
    else:
        io_dt = f32
        out_dt = f32

    SL = _token_slices(C)
    SOFF = [0]
    for w in SL:
        SOFF.append(SOFF[-1] + w)
    TN = len(SL)
    NT = C // 128        # token tiles for GEMM2

    nc = bacc.Bacc("TRN2", target_bir_lowering=False, debug=False,
                   enable_asserts=False, num_devices=N_CORES)

    # Host-packed partition-major layouts: every DMA below moves full
    # contiguous per-partition rows.
    xT = nc.dram_tensor("xT", [128, KD * C], io_dt, kind="ExternalInput")
    w12 = nc.dram_tensor("w12", [128, NCH * KD * 128], io_dt,
                         kind="ExternalInput")
    w3 = nc.dram_tensor("w3", [128, KH * DIM], io_dt, kind="ExternalInput")
    gt = nc.dram_tensor("gt", [128, NT], f32, kind="ExternalInput")
    if with_b12:
        b1 = nc.dram_tensor("b1", [128, KH], f32, kind="ExternalInput")
        b2 = nc.dram_tensor("b2", [128, KH], f32, kind="ExternalInput")
    out = nc.dram_tensor("out", [C, DIM], out_dt, kind="ExternalOutput")

    silu = mybir.ActivationFunctionType.Silu
    ident = mybir.ActivationFunctionType.Copy

    with tile.TileContext(nc) as tc:
        with (
            tc.tile_pool(name="weights", bufs=1) as wpool,
            tc.tile_pool(name="tmp", bufs=4) as tpool,
            tc.tile_pool(name="ps_g1", bufs=4, space="PSUM") as pspool1,
            tc.tile_pool(name="ps_g2", bufs=4, space="PSUM") as pspool2,
        ):
            w12sb = wpool.tile([128, NCH, KD, 128], io_dt, tag="w12sb")
            xTsb = wpool.tile([128, KD * C], io_dt, tag="xTsb")
            w3sb = wpool.tile([128, KH, DIM], io_dt, tag="w3sb")
            gsb = wpool.tile([128, NT], f32, tag="gsb")
            hid = wpool.tile([128, KH, C], io_dt, tag="hid")
            ssb = wpool.tile([128, KH, 512], f32, tag="ssb")
            warm = wpool.tile([128, 384], mybir.dt.bfloat16, tag="warm")
            if with_b12:
                b1sb = wpool.tile([128, KH], f32, tag="b1sb")
                b2sb = wpool.tile([128, KH], f32, tag="b2sb")

            # ---- PE warm-up: ramp DVFS while the first DMAs are in flight
            nc.vector.memset(warm[:], 0)
            wp = pspool2.tile([128, DSLICE], f32, tag="pso",
                              name="wp")[:, :256]
            for i in range(WARM_N):
                nc.tensor.matmul(wp, warm[:, 256:384], warm[:, 0:256],
                                 start=(i == 0), stop=(i == WARM_N - 1))
            # consume so nothing upstream can be considered dead
            wsink = tpool.tile([128, 256], f32, tag="o")
            nc.vector.tensor_scalar_mul(wsink, wp, 0.0)

            # ---- input DMAs: two HWDGE rings (sync, scalar) in parallel,
            # few large transfers with fat descriptors (>=2KB/partition-row;
            # small rows are ~80ns/descriptor overhead-bound), ordered by
            # first consumption.
            def w12_dma(eng, c0, c1):
                eng.dma_start(w12sb[:, c0:c1, :, :],
                              w12[:, c0 * KD * 128:c1 * KD * 128])

            def xT_dma(eng, n, k0, k1):
                sw = SL[n]
                base = SOFF[n] * KD
                eng.dma_start(
                    xTsb[:, base + k0 * sw:base + k1 * sw],
                    xT[:, base + k0 * sw:base + k1 * sw])

            sy, sc = nc.sync, nc.scalar

            def w12k_dma(eng, c, k0, k1):
                eng.dma_start(w12sb[:, c, k0:k1, :],
                              w12[:, (c * KD + k0) * 128:(c * KD + k1) * 128])

            w12k_dma(sy, 0, 0, 2)
            xT_dma(sc, 0, 0, 2)
            xT_dma(sy, 0, 2, 4)
            w12k_dma(sc, 0, 2, KD)
            w12_dma(sy, 1, 3)
            xT_dma(sc, 0, 4, KD)
            w12_dma(sy, 5, 7)
            w12_dma(sc, 3, 5)
            w12_dma(sy, 9, NCH)
            w12_dma(sc, 7, 9)
            sy.dma_start(w3sb[:], w3[:])
            for n in range(1, TN):
                xT_dma(sc, n, 0, KD)
            sy.dma_start(gsb[:], gt[:])
            if with_b12:
                sc.dma_start(b1sb[:], b1[:])
                sc.dma_start(b2sb[:], b2[:])

            def _gemm2_tile(t):
                tsl = slice(t * 128, (t + 1) * 128)
                for d in range(DIM // DSLICE):
                    pso = pspool2.tile([128, DSLICE], f32, tag="pso")
                    dsl = slice(d * DSLICE, (d + 1) * DSLICE)
                    for k in range(KH):
                        if k == KH - 1 and H % 128:
                            hh = H % 128
                            nc.tensor.matmul(
                                pso, hid[0:hh, k, tsl], w3sb[0:hh, k, dsl],
                                start=(k == 0), stop=True)
                        else:
                            nc.tensor.matmul(
                                pso, hid[:, k, tsl], w3sb[:, k, dsl],
                                start=(k == 0), stop=(k == KH - 1))
                    o = tpool.tile([128, DSLICE], out_dt, tag="o")
                    # scalar, not vector: a 128-part DVE read of PSUM
                    # steals PE accumulate bandwidth (k0/k1 ran at 318ns)
                    nc.scalar.activation(o, pso, ident, scale=gsb[:, t:t + 1])
                    nc.sync.dma_start(out[tsl, dsl], o)

            t_emitted = 0
            for n in range(TN):
                w = SL[n]
                ns = slice(SOFF[n], SOFF[n] + w)
                xbase = SOFF[n] * KD
                # GEMM1: 11 unpadded chunks; chunks 0..5(:64) are x1,
                # chunks 5(64:)..10 are x2, offset by 64 partitions.
                for c in range(NCH):
                    ps = pspool1.tile([128, 512], f32, tag="g1ps",
                                      name="g1ps")[:, :w]
                    for k in range(KD):
                        nc.tensor.matmul(
                            ps, w12sb[:, c, k, :],
                            xTsb[:, xbase + k * w:xbase + (k + 1) * w],
                            start=(k == 0), stop=(k == KD - 1))
                    if c < NP:
                        if with_b12:
                            nc.scalar.activation(ssb[:, c, :w], ps, silu,
                                                 bias=b1sb[:, c:c + 1])
                        else:
                            nc.scalar.activation(ssb[:, c, :w], ps, silu)
                    elif c == NP:
                        # lower 64: x1 tail; upper 64: x2 cols 0..63
                        if with_b12:
                            nc.scalar.activation(ssb[0:64, NP, :w], ps[0:64],
                                                 silu, bias=b1sb[0:64, NP:NP + 1])
                            nc.vector.tensor_scalar_add(
                                ps[64:128], ps[64:128], b2sb[64:128, 0:1])
                        else:
                            nc.scalar.activation(ssb[0:64, NP, :w], ps[0:64],
                                                 silu)
                        nc.vector.tensor_mul(
                            out=hid[0:64, 0, ns], in0=ssb[0:64, 0, :w],
                            in1=ps[64:128])
                    else:
                        p_lo = c - NP - 1   # pair completing its upper half
                        p_hi = c - NP       # pair starting its lower half
                        if with_b12:
                            nc.vector.tensor_scalar_add(
                                ps, ps, b2sb[:, c - NP:c - NP + 1])
                        nc.vector.tensor_mul(
                            out=hid[64:128, p_lo, ns],
                            in0=ssb[64:128, p_lo, :w], in1=ps[0:64])
                        nc.vector.tensor_mul(
                            out=hid[0:64, p_hi, ns],
                            in0=ssb[0:64, p_hi, :w], in1=ps[64:128])

                # GEMM2 lags one slice behind GEMM1 so the PE never waits
                # on the SwiGLU chain at a slice seam.
                for t in range(t_emitted, SOFF[n] // 128):
                    _gemm2_tile(t)
                t_emitted = SOFF[n] // 128
            for t in range(t_emitted, NT):
                _gemm2_tile(t)

    nc.compile()
    return nc


def _np_io_dtype(mm_dt):
    if mm_dt == "bf16":
        import ml_dtypes

        return np.dtype(ml_dtypes.bfloat16)
    return np.dtype(np.float32)


def kernel(hidden_states, top_k_weights, W12, b12, W3, b3, top_k_index):
    global LAST_RESULTS
    from concourse.bass_utils import run_bass_kernel_spmd

    hs = np.asarray(hidden_states, dtype=np.float32)
    wts = np.asarray(top_k_weights, dtype=np.float32)
    idx = np.asarray(top_k_index)
    W12n = np.asarray(W12, dtype=np.float32)
    b12n = np.asarray(b12, dtype=np.float32)
    W3n = np.asarray(W3, dtype=np.float32)
    b3n = np.asarray(b3, dtype=np.float32)

    T = hs.shape[0]
    mm_dt = MM_DT_NAME
    io_np = _np_io_dtype(mm_dt)

    # ---- routing on host ----
    gates = np.zeros((E, T), np.float32)
    for k in range(TOPK):
        np.add.at(gates, (idx[:, k], np.arange(T)), wts[:, k])
    tok = [np.nonzero((idx == e).any(axis=1))[0] for e in range(E)]
    maxlen = max(256, max(len(t) for t in tok))
    C = ((maxlen + 127) // 128) * 128
    NT = C // 128

    with_b12 = bool(np.any(b12n))
    key = (C, mm_dt, with_b12)
    if key not in _BUILD_CACHE:
        _BUILD_CACHE[key] = _build_program(C, mm_dt, with_b12)
    nc = _BUILD_CACHE[key]

    # ---- per-core inputs ----
    in_maps = []
    for e in range(E):
        te = tok[e]
        ne = len(te)
        X = np.zeros((C, DIM), np.float32)
        X[:ne] = hs[te]
        # per-slice [128, KD, w] partition-major packs, concatenated
        blocks = []
        off = 0
        for w in _token_slices(C):
            blk = X[off:off + w].reshape(w, KD, 128).transpose(2, 1, 0)
            blocks.append(np.ascontiguousarray(blk).reshape(128, -1))
            off += w
        xTp = np.concatenate(blocks, axis=1).astype(io_np, copy=False)

        # w12: [DIM, 2H] -> [128, (c, k, m)] with c over 11 output chunks
        w12p = np.ascontiguousarray(
            W12n[e].reshape(KD, 128, NCH, 128).transpose(1, 2, 0, 3)
        ).reshape(128, -1)

        # w3: [H, DIM] zero-padded to 6*128 rows -> [128, (c, d)]
        w3p = np.zeros((KH * 128, DIM), np.float32)
        w3p[:H] = W3n[e]
        w3p = np.ascontiguousarray(
            w3p.reshape(KH, 128, DIM).transpose(1, 0, 2)).reshape(128, -1)

        g = np.zeros((C,), np.float32)
        g[:ne] = gates[e, te]
        gtile = np.ascontiguousarray(g.reshape(NT, 128).T)

        m = {
            "xT": xTp,
            "w12": w12p.astype(io_np, copy=False),
            "w3": w3p.astype(io_np, copy=False),
            "gt": gtile,
        }
        if with_b12:
            b1p = np.zeros((128, KH), np.float32)
            b2p = np.zeros((128, KH), np.float32)
            for c in range(KH):
                n1 = min(128, H - c * 128)
                b1p[:n1, c] = b12n[e][c * 128:c * 128 + n1]
                for p in range(128):
                    j = c * 128 + p - 64
                    if 0 <= j < H:
                        b2p[p, c] = b12n[e][H + j]
            m["b1"] = np.ascontiguousarray(b1p)
            m["b2"] = np.ascontiguousarray(b2p)
        in_maps.append(m)

    trace = bool(os.environ.get("KERNEL_TRACE"))
    kw = {}
    if trace:
        _ensure_ntff_hook()
        kw = {"trace_cores": list(range(N_CORES)), "stitch_traces": False}
    res = run_bass_kernel_spmd(nc, in_maps, list(range(N_CORES)), trace=trace, **kw)
    LAST_RESULTS = res

    # ---- combine on host ----
    out = np.zeros((T, DIM), np.float32)
    for e in range(E):
        te = tok[e]
        out[te] += res.results[e]["out"][:len(te)].astype(np.float32)
    if np.any(b3n):
        out += gates.T @ b3n
    return out


# revision 33
# speedup vs baseline: 1.0625x; 1.0307x over previous
"""MoE (DeepSeek-style naive top-k routing + per-expert SwiGLU) on 8 Trainium2 cores.

Strategy: expert parallelism with host-side token dispatch/combine.
  - Host computes the routing (top_k_index/top_k_weights -> per-expert token
    lists + combine gates), gathers each expert's tokens into a padded
    capacity-C buffer, and hands core e exactly expert e's weights + tokens.
  - Each core runs dense SwiGLU over its C tokens in bf16 (1 col/cycle on the
    PE, same rate as f32r, but half the HBM traffic):
        Y^T = W12^T @ X^T           (GEMM1, contraction over DIM=1024,
                                     11 unpadded 128-row output chunks)
        hidden = silu(x1) * x2      (partition-shifted 64-wide muls: x2
                                     chunks are offset by 64 partitions
                                     from x1 chunks since H=704=5.5*128)
        out = hidden^T' @ W3        (GEMM2, contraction over H in 6 chunks,
                                     last chunk 64 rows)
    with the per-token combine gate folded into the PSUM->SBUF copy of the
    GEMM2 result.
  - Host scatter-adds the 8 per-expert partial outputs into the [T, DIM] out.

Ramp optimizations: DMA triggers alternate between the two HW-DGE rings
(Sync + Activation engines) so descriptor generation is not serialized; the
first w12/xT chunks are split small so the PE starts ~6us earlier; a short
burst of warm-up matmuls on zeroed SBUF ramps the PE clock (DVFS) while the
first DMAs are still in flight.
"""

import os
import sys

for _p in ("/opt/trn_rl_repo",):
    if _p not in sys.path:
        sys.path.insert(0, _p)

import numpy as np

E = 8
DIM = 1024
H = 704
TOPK = 2
KD = DIM // 128      # contraction tiles for GEMM1
NCH = (2 * H) // 128  # GEMM1 output chunks (11, unpadded)
KH = (H + 127) // 128  # GEMM2 contraction chunks (6, last is 64 rows)
NP = H // 128         # full swiglu pairs (5); pair 5 is the 64-wide tail
DSLICE = 512          # DIM slice width for GEMM2
N_CORES = 8
WARM_N = 14           # PE warm-up matmuls (DVFS ramp) before data lands
# Filler warm matmuls at the known early-DMA stall points: the PE clock
# drops after any idle gap (hysteresis), so idling while xT chunks land
# costs ~2x on the next ~15 matmuls. Filler keeps the clock up; it is
# sized to the slow-core stall so on-time cores lose nothing that the
# slowest core doesn't already lose.
FILL_K2 = 9           # before c0 k2 (waits xT k2-3)
FILL_K4 = 13          # before c0 k4 (waits xT k4-7)
FILL_C1 = 3           # before c1 (waits w12 c1-2)


def _token_slices(C):
    """Split C (mult of 128) into GEMM1 slice widths <=512, each >=256
    where possible (f32r runs 1 cyc/row only at N>=256)."""
    out = []
    rem = C
    while rem > 640:
        out.append(512)
        rem -= 512
    if rem > 512:
        a = (rem // 2 + 127) // 128 * 128
        out += [a, rem - a]
    elif rem:
        out.append(rem)
    return out


MM_DT_NAME = os.environ.get("KERNEL_MM_DT", "bf16")  # f32 | f32r | bf16

_BUILD_CACHE = {}
LAST_RESULTS = None  # test harness reads exec_time_ns etc. from here


def _ensure_ntff_hook():
    """Profiling-only: register the ctypes NTFF hook (antenv.axon_hooks is
    not shipped in this container) and keep profile post-processing local."""
    import types

    import concourse.bass_utils as bu

    try:
        from antenv.axon_hooks import get_axon_ntff_profile_hook  # noqa: F401
    except ImportError:
        try:
            from trn_agent_boot.trn_boot import _ntff_profile_via_ctypes

            hook = _ntff_profile_via_ctypes("/opt/axon/libaxon_pjrt.so")
        except Exception:
            hook = None
        mod = types.ModuleType("antenv.axon_hooks")
        mod.get_axon_ntff_profile_hook = lambda: hook
        mod.set_axon_ntff_profile_hook = lambda h: None
        sys.modules["antenv.axon_hooks"] = mod
        import antenv

        antenv.axon_hooks = mod
    # keep artifacts local — no bucket in this container
    bu.upload_artifacts = lambda tmpdir: f"local://{tmpdir}"


def _install_drain_patch():
    """walrus 2026-05 rejects >1 sem wait on CTRL-class (Drain/NoOp) SP
    instructions; respell Tile's tail drain as a chain of 1-wait NOPs."""
    import concourse.mybir as mybir
    import concourse.tile as tile
    from concourse.tile import ScopedClock

    if getattr(tile.TileContext, "_drain_patch_installed", False):
        return

    def _patched(self, tick_clock, wait_clock):
        nc = self.nc
        nop_inst = nc.sync.nop(nofuse=True, hint="drain_waits")
        wait_clock.add_sem_waits(
            nop_inst.ins, ScopedClock({None: tick_clock.global_clock})
        )
        waits = list(nop_inst.ins.sync_info.on_wait or [])
        if len(waits) > 1:
            nop_inst.ins.sync_info.on_wait = waits[:1]
            for w in waits[1:]:
                extra = nc.sync.nop(nofuse=True, hint="drain_waits")
                extra.ins.sync_info = mybir.SyncInfo(on_wait=[w], on_update=[])
        nc.sync.drain()
        nc.all_engine_barrier()
        assert self.sems is not None
        popped = nc._tile_sem_poison_stack.pop()
        assert popped is self._sem_poison
        nc.clear_and_free_semaphores(list(self.sems.allocated().values()))
        nc.all_engine_barrier()

    tile.TileContext._drain_and_barrier = _patched
    tile.TileContext._drain_patch_installed = True


def _build_program(C, mm_dt, with_b12):
    """Build the single-core Bass program (SPMD: same program, per-core data)."""
    import concourse.bacc as bacc
    import concourse.bass as bass  # noqa: F401
    import concourse.mybir as mybir
    import concourse.tile as tile

    f32 = mybir.dt.float32
    if mm_dt == "bf16":
        io_dt = mybir.dt.bfloat16
        out_dt = mybir.dt.bfloat16
    elif mm_dt == "f32r":
        io_dt = mybir.dt.float32r
        out_dt = f32
    else:
        io_dt = f32
        out_dt = f32

    SL = _token_slices(C)
    SOFF = [0]
    for w in SL:
        SOFF.append(SOFF[-1] + w)
    TN = len(SL)
    NT = C // 128        # token tiles for GEMM2

    nc = bacc.Bacc("TRN2", target_bir_lowering=False, debug=False,
                   enable_asserts=False, num_devices=N_CORES)

    # Host-packed partition-major layouts: every DMA below moves full
    # contiguous per-partition rows.
    xT = nc.dram_tensor("xT", [128, KD * C], io_dt, kind="ExternalInput")
    w12 = nc.dram_tensor("w12", [128, NCH * KD * 128], io_dt,
                         kind="ExternalInput")
    w3 = nc.dram_tensor("w3", [128, KH * DIM], io_dt, kind="ExternalInput")
    gt = nc.dram_tensor("gt", [128, NT], f32, kind="ExternalInput")
    if with_b12:
        b1 = nc.dram_tensor("b1", [128, KH], f32, kind="ExternalInput")
        b2 = nc.dram_tensor("b2", [128, KH], f32, kind="ExternalInput")
    out = nc.dram_tensor("out", [C, DIM], out_dt, kind="ExternalOutput")

    silu = mybir.ActivationFunctionType.Silu
    ident = mybir.ActivationFunctionType.Copy

    with tile.TileContext(nc) as tc:
        with (
            tc.tile_pool(name="weights", bufs=1) as wpool,
            tc.tile_pool(name="tmp", bufs=4) as tpool,
            tc.tile_pool(name="ps_g1", bufs=4, space="PSUM") as pspool1,
            tc.tile_pool(name="ps_g2", bufs=4, space="PSUM") as pspool2,
        ):
            w12sb = wpool.tile([128, NCH, KD, 128], io_dt, tag="w12sb")
            xTsb = wpool.tile([128, KD * C], io_dt, tag="xTsb")
            w3sb = wpool.tile([128, KH, DIM], io_dt, tag="w3sb")
            gsb = wpool.tile([128, NT], f32, tag="gsb")
            hid = wpool.tile([128, KH, C], io_dt, tag="hid")
            ssb = wpool.tile([128, KH, 512], f32, tag="ssb")
            warm = wpool.tile([128, 384], mybir.dt.bfloat16, tag="warm")
            if with_b12:
                b1sb = wpool.tile([128, KH], f32, tag="b1sb")
                b2sb = wpool.tile([128, KH], f32, tag="b2sb")

            # ---- PE warm-up: ramp DVFS while the first DMAs are in flight
            nc.vector.memset(warm[:], 0)
            wp = pspool2.tile([128, DSLICE], f32, tag="pso",
                              name="wp")[:, :256]

            def _filler(m):
                for _ in range(m):
                    nc.tensor.matmul(wp, warm[:, 256:384], warm[:, 0:256],
                                     start=True, stop=True)

            _filler(WARM_N)

            # ---- input DMAs: two HWDGE rings (sync, scalar) in parallel,
            # few large transfers with fat descriptors (>=2KB/partition-row;
            # small rows are ~80ns/descriptor overhead-bound), ordered by
            # first consumption.
            def w12_dma(eng, c0, c1):
                eng.dma_start(w12sb[:, c0:c1, :, :],
                              w12[:, c0 * KD * 128:c1 * KD * 128])

            def xT_dma(eng, n, k0, k1):
                sw = SL[n]
                base = SOFF[n] * KD
                eng.dma_start(
                    xTsb[:, base + k0 * sw:base + k1 * sw],
                    xT[:, base + k0 * sw:base + k1 * sw])

            sy, sc = nc.sync, nc.scalar

            def w12k_dma(eng, c, k0, k1):
                eng.dma_start(w12sb[:, c, k0:k1, :],
                              w12[:, (c * KD + k0) * 128:(c * KD + k1) * 128])

            w12k_dma(sy, 0, 0, 2)
            xT_dma(sc, 0, 0, 2)
            xT_dma(sy, 0, 2, 4)
            w12k_dma(sc, 0, 2, KD)
            w12_dma(sy, 1, 3)
            xT_dma(sc, 0, 4, KD)
            w12_dma(sy, 5, 7)
            w12_dma(sc, 3, 5)
            w12_dma(sy, 9, NCH)
            w12_dma(sc, 7, 9)
            sy.dma_start(w3sb[:], w3[:])
            for n in range(1, TN):
                xT_dma(sc, n, 0, KD)
            sy.dma_start(gsb[:], gt[:])
            if with_b12:
                sc.dma_start(b1sb[:], b1[:])
                sc.dma_start(b2sb[:], b2[:])

            def _gemm2_tile(t):
                tsl = slice(t * 128, (t + 1) * 128)
                for d in range(DIM // DSLICE):
                    pso = pspool2.tile([128, DSLICE], f32, tag="pso")
                    dsl = slice(d * DSLICE, (d + 1) * DSLICE)
                    for k in range(KH):
                        if k == KH - 1 and H % 128:
                            hh = H % 128
                            nc.tensor.matmul(
                                pso, hid[0:hh, k, tsl], w3sb[0:hh, k, dsl],
                                start=(k == 0), stop=True)
                        else:
                            nc.tensor.matmul(
                                pso, hid[:, k, tsl], w3sb[:, k, dsl],
                                start=(k == 0), stop=(k == KH - 1))
                    o = tpool.tile([128, DSLICE], out_dt, tag="o")
                    # scalar, not vector: a 128-part DVE read of PSUM
                    # steals PE accumulate bandwidth (k0/k1 ran at 318ns)
                    nc.scalar.activation(o, pso, ident, scale=gsb[:, t:t + 1])
                    nc.sync.dma_start(out[tsl, dsl], o)

            t_emitted = 0
            for n in range(TN):
                w = SL[n]
                ns = slice(SOFF[n], SOFF[n] + w)
                xbase = SOFF[n] * KD
                # GEMM1: 11 unpadded chunks; chunks 0..5(:64) are x1,
                # chunks 5(64:)..10 are x2, offset by 64 partitions.
                for c in range(NCH):
                    if n == 0 and c == 1:
                        _filler(FILL_C1)
                    ps = pspool1.tile([128, 512], f32, tag="g1ps",
                                      name="g1ps")[:, :w]
                    for k in range(KD):
                        if n == 0 and c == 0:
                            if k == 2:
                                _filler(FILL_K2)
                            elif k == 4:
                                _filler(FILL_K4)
                        nc.tensor.matmul(
                            ps, w12sb[:, c, k, :],
                            xTsb[:, xbase + k * w:xbase + (k + 1) * w],
                            start=(k == 0), stop=(k == KD - 1))
                    if c < NP:
                        if with_b12:
                            nc.scalar.activation(ssb[:, c, :w], ps, silu,
                                                 bias=b1sb[:, c:c + 1])
                        else:
                            nc.scalar.activation(ssb[:, c, :w], ps, silu)
                    elif c == NP:
                        # lower 64: x1 tail; upper 64: x2 cols 0..63
                        if with_b12:
                            nc.scalar.activation(ssb[0:64, NP, :w], ps[0:64],
                                                 silu, bias=b1sb[0:64, NP:NP + 1])
                            nc.vector.tensor_scalar_add(
                                ps[64:128], ps[64:128], b2sb[64:128, 0:1])
                        else:
                            nc.scalar.activation(ssb[0:64, NP, :w], ps[0:64],
                                                 silu)
                        nc.vector.tensor_mul(
                            out=hid[0:64, 0, ns], in0=ssb[0:64, 0, :w],
                            in1=ps[64:128])
                    else:
                        p_lo = c - NP - 1   # pair completing its upper half
                        p_hi = c - NP       # pair starting its lower half
                        if with_b12:
                            nc.vector.tensor_scalar_add(
                                ps, ps, b2sb[:, c - NP:c - NP + 1])
                        nc.vector.tensor_mul(
                            out=hid[64:128, p_lo, ns],
                            in0=ssb[64:128, p_lo, :w], in1=ps[0:64])
                        nc.vector.tensor_mul(
                            out=hid[0:64, p_hi, ns],
                            in0=ssb[0:64, p_hi, :w], in1=ps[64:128])

                if n == 0:
                    # consume the warm psum (frees its PSUM buf before G2)
                    wsink = tpool.tile([128, 256], f32, tag="o")
                    nc.vector.tensor_scalar_mul(wsink, wp, 0.0)
                # GEMM2 lags one slice behind GEMM1 so the PE never waits
                # on the SwiGLU chain at a slice seam.
                for t in range(t_emitted, SOFF[n] // 128):
                    _gemm2_tile(t)
                t_emitted = SOFF[n] // 128
            for t in range(t_emitted, NT):
                _gemm2_tile(t)

    nc.compile()
    return nc


def _np_io_dtype(mm_dt):
    if mm_dt == "bf16":
        import ml_dtypes

        return np.dtype(ml_dtypes.bfloat16)
    return np.dtype(np.float32)


def kernel(hidden_states, top_k_weights, W12, b12, W3, b3, top_k_index):
    global LAST_RESULTS
    from concourse.bass_utils import run_bass_kernel_spmd

    hs = np.asarray(hidden_states, dtype=np.float32)
    wts = np.asarray(top_k_weights, dtype=np.float32)
    idx = np.asarray(top_k_index)
    W12n = np.asarray(W12, dtype=np.float32)
    b12n = np.asarray(b12, dtype=np.float32)
    W3n = np.asarray(W3, dtype=np.float32)
    b3n = np.asarray(b3, dtype=np.float32)

    T = hs.shape[0]
    mm_dt = MM_DT_NAME
    io_np = _np_io_dtype(mm_dt)

    # ---- routing on host ----
    gates = np.zeros((E, T), np.float32)
    for k in range(TOPK):
        np.add.at(gates, (idx[:, k], np.arange(T)), wts[:, k])
    tok = [np.nonzero((idx == e).any(axis=1))[0] for e in range(E)]
    maxlen = max(256, max(len(t) for t in tok))
    C = ((maxlen + 127) // 128) * 128
    NT = C // 128

    with_b12 = bool(np.any(b12n))
    key = (C, mm_dt, with_b12)
    if key not in _BUILD_CACHE:
        _BUILD_CACHE[key] = _build_program(C, mm_dt, with_b12)
    nc = _BUILD_CACHE[key]

    # ---- per-core inputs ----
    in_maps = []
    for e in range(E):
        te = tok[e]
        ne = len(te)
        X = np.zeros((C, DIM), np.float32)
        X[:ne] = hs[te]
        # per-slice [128, KD, w] partition-major packs, concatenated
        blocks = []
        off = 0
        for w in _token_slices(C):
            blk = X[off:off + w].reshape(w, KD, 128).transpose(2, 1, 0)
            blocks.append(np.ascontiguousarray(blk).reshape(128, -1))
            off += w
        xTp = np.concatenate(blocks, axis=1).astype(io_np, copy=False)

        # w12: [DIM, 2H] -> [128, (c, k, m)] with c over 11 output chunks
        w12p = np.ascontiguousarray(
            W12n[e].reshape(KD, 128, NCH, 128).transpose(1, 2, 0, 3)
        ).reshape(128, -1)

        # w3: [H, DIM] zero-padded to 6*128 rows -> [128, (c, d)]
        w3p = np.zeros((KH * 128, DIM), np.float32)
        w3p[:H] = W3n[e]
        w3p = np.ascontiguousarray(
            w3p.reshape(KH, 128, DIM).transpose(1, 0, 2)).reshape(128, -1)

        g = np.zeros((C,), np.float32)
        g[:ne] = gates[e, te]
        gtile = np.ascontiguousarray(g.reshape(NT, 128).T)

        m = {
            "xT": xTp,
            "w12": w12p.astype(io_np, copy=False),
            "w3": w3p.astype(io_np, copy=False),
            "gt": gtile,
        }
        if with_b12:
            b1p = np.zeros((128, KH), np.float32)
            b2p = np.zeros((128, KH), np.float32)
            for c in range(KH):
                n1 = min(128, H - c * 128)
                b1p[:n1, c] = b12n[e][c * 128:c * 128 + n1]
                for p in range(128):
                    j = c * 128 + p - 64
                    if 0 <= j < H:
                        b2p[p, c] = b12n[e][H + j]
            m["b1"] = np.ascontiguousarray(b1p)
            m["b2"] = np.ascontiguousarray(b2p)
        in_maps.append(m)

    trace = bool(os.environ.get("KERNEL_TRACE"))
    kw = {}
    if trace:
        _ensure_ntff_hook()
        kw = {"trace_cores": list(range(N_CORES)), "stitch_traces": False}
    res = run_bass_kernel_spmd(nc, in_maps, list(range(N_CORES)), trace=trace, **kw)
    LAST_RESULTS = res

    # ---- combine on host ----
    out = np.zeros((T, DIM), np.float32)
    for e in range(E):
        te = tok[e]
        out[te] += res.results[e]["out"][:len(te)].astype(np.float32)
    if np.any(b3n):
        out += gates.T @ b3n
    return out
